# revision 7
# baseline (speedup 1.0000x reference)
"""Trainium2 Bass kernel for nn_DarcyResidual (P=256, B=128, 8 NeuronCores).

Math (reference):
    a = (x0 + 1.5) / 0.2 = 5*(x0 + 1.5)
    p = (x1 + 0.9) / 115
    residual = -a*(p_d00 + p_d11) - a_d0*p_d0 - a_d1*p_d1 - 1
with 2nd-order central differences inside, 2nd-order one-sided at borders,
grid spacing h = 1/256 on both axes.

Folding all affine offsets and h / 115 / 5 scale factors:
    residual = -G * [ (X0 + 1.5)*U4 + S1*R1 + C1a*C1p ] - 1
      G   = 5 / (460 h^2)
      U4  = 4*(rowD2raw(X1) + colD2raw(X1))     (h^2-scaled raw 2nd diffs)
      R1  = rowD1raw(X1), S1 = rowD1raw(X0)      (2h-scaled raw 1st diffs)
      C1p = colD1raw(X1), C1a = colD1raw(X0)

Layout per core (16 images): SBUF [partition p = row-within-128-block,
free = (k row-block 2, image b, col j 256)].  Row-direction (d0) derivatives
are computed on TensorE as banded-matrix matmuls (boundary rows included in
the matrices; the -2*I diagonal term of the combined row+col Laplacian is
folded into the row matrix).  Column-direction (d1) stencils are shifted-AP
DVE/GPSIMD elementwise ops; image-border columns j=0,255 are handled by a
small gather pipeline with one-sided stencils.
"""

import numpy as np

P = 256
B = 128
NCORES = 8
BPC = B // NCORES          # images per core = 16
CHUNKS = 4                 # images per chunk = 4
BCH = BPC // CHUNKS        # = 4
FCH = 2 * BCH * P          # chunk free size = 2048
GAMMA = 5.0 * 65536.0 / 460.0

_cache = {}


def _weights():
    """[128, 12, 128] stacked lhsT blocks for the row-stencil matmuls."""
    D1 = np.zeros((P, P), dtype=np.float64)
    for i in range(1, P - 1):
        D1[i, i - 1] = -1.0
        D1[i, i + 1] = 1.0
    D1[0, 0:3] = [-3.0, 4.0, -1.0]
    D1[P - 1, P - 3:P] = [1.0, -4.0, 3.0]

    D2 = np.zeros((P, P), dtype=np.float64)
    for i in range(1, P - 1):
        D2[i, i - 1] = 1.0
        D2[i, i] = -2.0
        D2[i, i + 1] = 1.0
    D2[0, 0:4] = [2.0, -5.0, 4.0, -1.0]
    D2[P - 1, P - 4:P] = [-1.0, 4.0, -5.0, 2.0]

    WR2 = 4.0 * (D2 - 2.0 * np.eye(P))   # main pipeline (col -2I folded here)
    WR2E = 4.0 * D2                      # edge pipeline (col stencil complete)

    wts = np.zeros((128, 12, 128), dtype=np.float32)
    for m in range(2):
        for kb in range(2):
            i = m * 2 + kb
            blk = lambda W: W[m * 128:(m + 1) * 128, kb * 128:(kb + 1) * 128].T
            wts[:, i, :] = blk(D1)
            wts[:, 4 + i, :] = blk(WR2)
            wts[:, 8 + i, :] = blk(WR2E)
    return wts


def _build_program():
    from concourse import bacc
    import concourse.mybir as mybir
    from concourse.tile import TileContext

    f32 = mybir.dt.float32
    ADD = mybir.AluOpType.add
    SUB = mybir.AluOpType.subtract
    MUL = mybir.AluOpType.mult

    nc = bacc.Bacc("TRN2", target_bir_lowering=False, debug=False,
                   num_devices=NCORES)
    xin = nc.dram_tensor("xin", [128, 2, 2, BPC, P], f32, kind="ExternalInput")
    wts = nc.dram_tensor("wts", [128, 12, 128], f32, kind="ExternalInput")
    yout = nc.dram_tensor("yout", [128, 2, BPC, P], f32, kind="ExternalOutput")

    with TileContext(nc) as tc:
        with (
            tc.tile_pool(name="const", bufs=1) as cpool,
            tc.tile_pool(name="edge", bufs=1) as epool,
            tc.tile_pool(name="work", bufs=2) as pool,
        ):
            wt = cpool.tile([128, 12, 128], f32)
            nc.sync.dma_start(out=wt[:], in_=wts[:])

            def W(i):
                return wt[:, i, :]

            # ---------------- edge pipeline (output cols j=0 and j=255) ----
            X0e = epool.tile([128, 2, BPC, 8], f32)
            X1e = epool.tile([128, 2, BPC, 8], f32)
            nc.sync.dma_start(out=X0e[:, :, :, 0:4], in_=xin[:, 0, :, :, 0:4])
            nc.sync.dma_start(out=X0e[:, :, :, 4:8], in_=xin[:, 0, :, :, P - 4:P])
            nc.sync.dma_start(out=X1e[:, :, :, 0:4], in_=xin[:, 1, :, :, 0:4])
            nc.sync.dma_start(out=X1e[:, :, :, 4:8], in_=xin[:, 1, :, :, P - 4:P])

            def ecol(t, c):
                return (t.rearrange("p k b c -> p (k b) c")[:, :, c:c + 1]
                        .rearrange("p n o -> p (n o)"))

            X0ef = X0e.rearrange("p k b c -> p (k b c)")
            X1ef = X1e.rearrange("p k b c -> p (k b c)")

            with tc.tile_pool(name="psum_e", bufs=1, space="PSUM") as ppe:
                R2e = ppe.tile([128, 2, BPC, 8], f32)
                R1e = ppe.tile([128, 2, BPC, 8], f32)
                S1e = ppe.tile([128, 2, BPC, 8], f32)
                R2ef = R2e.rearrange("p k b c -> p (k b c)")
                R1ef = R1e.rearrange("p k b c -> p (k b c)")
                S1ef = S1e.rearrange("p k b c -> p (k b c)")
                for m in range(2):
                    osl = slice(m * 128, (m + 1) * 128)
                    for kb in range(2):
                        isl = slice(kb * 128, (kb + 1) * 128)
                        st, sp = kb == 0, kb == 1
                        nc.tensor.matmul(R1ef[:, osl], W(m * 2 + kb),
                                         X1ef[:, isl], start=st, stop=sp)
                        nc.tensor.matmul(S1ef[:, osl], W(m * 2 + kb),
                                         X0ef[:, isl], start=st, stop=sp)
                        nc.tensor.matmul(R2ef[:, osl], W(8 + m * 2 + kb),
                                         X1ef[:, isl], start=st, stop=sp)

                def et(name):
                    return epool.tile([128, 2 * BPC], f32, name=name, tag=name)

                # forward / mirrored first differences of the 4 border cols
                a1, b1, c1 = et("a1"), et("b1"), et("c1")
                am, bm, cm = et("am"), et("bm"), et("cm")
                a0, b0 = et("a0"), et("b0")
                a0m, b0m = et("a0m"), et("b0m")
                nc.vector.tensor_sub(a1[:], ecol(X1e, 1), ecol(X1e, 0))
                nc.vector.tensor_sub(b1[:], ecol(X1e, 2), ecol(X1e, 1))
                nc.vector.tensor_sub(c1[:], ecol(X1e, 3), ecol(X1e, 2))
                nc.vector.tensor_sub(am[:], ecol(X1e, 7), ecol(X1e, 6))
                nc.vector.tensor_sub(bm[:], ecol(X1e, 6), ecol(X1e, 5))
                nc.vector.tensor_sub(cm[:], ecol(X1e, 5), ecol(X1e, 4))
                nc.vector.tensor_sub(a0[:], ecol(X0e, 1), ecol(X0e, 0))
                nc.vector.tensor_sub(b0[:], ecol(X0e, 2), ecol(X0e, 1))
                nc.vector.tensor_sub(a0m[:], ecol(X0e, 7), ecol(X0e, 6))
                nc.vector.tensor_sub(b0m[:], ecol(X0e, 6), ecol(X0e, 5))

                # one-sided raw stencils
                q0, q1 = et("q0"), et("q1")
                C2e0, C2e1 = et("C2e0"), et("C2e1")
                C1pe0, C1pe1 = et("C1pe0"), et("C1pe1")
                C1ae0, C1ae1 = et("C1ae0"), et("C1ae1")
                stt = nc.vector.scalar_tensor_tensor
                stt(q0[:], b1[:], 3.0, c1[:], MUL, SUB)      # 3b - c
                stt(C2e0[:], a1[:], -2.0, q0[:], MUL, ADD)   # -2a + 3b - c
                stt(q1[:], bm[:], -3.0, cm[:], MUL, ADD)     # -3b + c
                stt(C2e1[:], am[:], 2.0, q1[:], MUL, ADD)    # 2a - 3b + c
                stt(C1pe0[:], a1[:], 3.0, b1[:], MUL, SUB)   # 3a - b
                stt(C1pe1[:], am[:], 3.0, bm[:], MUL, SUB)
                stt(C1ae0[:], a0[:], 3.0, b0[:], MUL, SUB)
                stt(C1ae1[:], a0m[:], 3.0, b0m[:], MUL, SUB)

                Scpe = epool.tile([128, 2, BPC, 8], f32)
                nc.scalar.copy(out=Scpe.rearrange("p k b c -> p (k b c)"),
                               in_=S1ef[:])

                rese = [None, None]
                for e, (C2e, C1pe, C1ae, ec) in enumerate(
                    ((C2e0, C1pe0, C1ae0, 0), (C2e1, C1pe1, C1ae1, 7))
                ):
                    U4e, tme = et(f"U4e{e}"), et(f"tme{e}")
                    t2e, t3e = et(f"t2e{e}"), et(f"t3e{e}")
                    res_e = epool.tile([128, 2 * BPC], f32, tag=f"rese{e}")
                    stt(U4e[:], C2e[:], 4.0, ecol(R2e, ec), MUL, ADD)
                    stt(tme[:], ecol(X0e, ec), 1.5, U4e[:], ADD, MUL)
                    nc.vector.tensor_mul(t2e[:], ecol(Scpe, ec), ecol(R1e, ec))
                    nc.vector.tensor_mul(t3e[:], C1ae[:], C1pe[:])
                    nc.vector.tensor_add(tme[:], tme[:], t2e[:])
                    nc.vector.tensor_add(tme[:], tme[:], t3e[:])
                    nc.scalar.activation(res_e[:], tme[:],
                                         mybir.ActivationFunctionType.Copy,
                                         bias=-1.0, scale=-GAMMA)
                    rese[e] = res_e

            # ---------------- main pipeline, 4 chunks of 4 images ----------
            with tc.tile_pool(name="psum", bufs=2, space="PSUM") as pp:
                for c in range(CHUNKS):
                    b0 = c * BCH
                    X0c = pool.tile([128, 2, BCH, P], f32, tag="x0")
                    X1c = pool.tile([128, 2, BCH, P], f32, tag="x1")
                    nc.sync.dma_start(out=X0c[:], in_=xin[:, 0, :, b0:b0 + BCH, :])
                    nc.sync.dma_start(out=X1c[:], in_=xin[:, 1, :, b0:b0 + BCH, :])
                    X0f = X0c.rearrange("p k b j -> p (k b j)")
                    X1f = X1c.rearrange("p k b j -> p (k b j)")

                    u2 = pool.tile([128, FCH], f32, tag="u2")
                    C1p = pool.tile([128, FCH], f32, tag="c1p")
                    C1a = pool.tile([128, FCH], f32, tag="c1a")
                    scp = pool.tile([128, FCH], f32, tag="scp")
                    tm = pool.tile([128, 2, BCH, P], f32, tag="tm")
                    tmf = tm.rearrange("p k b j -> p (k b j)")

                    # column stencils (interior cols; border cols overwritten)
                    nc.vector.tensor_add(u2[:, 1:FCH - 1], X1f[:, 2:FCH],
                                         X1f[:, 0:FCH - 2])
                    nc.gpsimd.tensor_sub(C1p[:, 1:FCH - 1], X1f[:, 2:FCH],
                                         X1f[:, 0:FCH - 2])
                    nc.gpsimd.tensor_sub(C1a[:, 1:FCH - 1], X0f[:, 2:FCH],
                                         X0f[:, 0:FCH - 2])

                    for m in range(2):
                        for bp in range(2):
                            lo = m * (BCH * P) + bp * (2 * P)
                            sl = slice(lo, lo + 2 * P)
                            R2 = pp.tile([128, 2 * P], f32, tag="r2")
                            R1 = pp.tile([128, 2 * P], f32, tag="r1")
                            S1 = pp.tile([128, 2 * P], f32, tag="s1")
                            for kb in range(2):
                                ilo = kb * (BCH * P) + bp * (2 * P)
                                isl = slice(ilo, ilo + 2 * P)
                                st, sp = kb == 0, kb == 1
                                nc.tensor.matmul(R1[:], W(m * 2 + kb),
                                                 X1f[:, isl], start=st, stop=sp)
                                nc.tensor.matmul(S1[:], W(m * 2 + kb),
                                                 X0f[:, isl], start=st, stop=sp)
                                nc.tensor.matmul(R2[:], W(4 + m * 2 + kb),
                                                 X1f[:, isl], start=st, stop=sp)
                            stt = nc.vector.scalar_tensor_tensor
                            # U4 in-place on u2
                            stt(u2[:, sl], u2[:, sl], 4.0, R2[:],
                                mybir.AluOpType.mult, mybir.AluOpType.add)
                            # tm = (X0 + 1.5) * U4
                            stt(tmf[:, sl], X0f[:, sl], 1.5, u2[:, sl],
                                mybir.AluOpType.add, mybir.AluOpType.mult)
                            nc.scalar.copy(out=scp[:, sl], in_=S1[:])
                            nc.vector.tensor_mul(scp[:, sl], scp[:, sl], R1[:])

                    # t3 in-place on C1a (gpsimd), then accumulate on tm
                    nc.gpsimd.tensor_mul(C1a[:], C1a[:], C1p[:])
                    nc.vector.tensor_add(tmf[:], tmf[:], scp[:])
                    nc.vector.tensor_add(tmf[:], tmf[:], C1a[:])

                    resv = tm.rearrange("p k b j -> p (k b) j")[:, :, 1:P - 1]
                    nc.scalar.activation(resv, resv,
                                         mybir.ActivationFunctionType.Copy,
                                         bias=-1.0, scale=-GAMMA)

                    # border columns from the edge pipeline
                    for e, j in ((0, 0), (1, P - 1)):
                        src = (rese[e].rearrange("p (k b o) -> p k b o",
                                                 k=2, o=1)[:, :, b0:b0 + BCH, :])
                        dst = tm[:, :, :, j:j + 1]
                        nc.scalar.copy(out=dst, in_=src)

                    nc.sync.dma_start(out=yout[:, :, b0:b0 + BCH, :], in_=tm[:])

    nc.compile()
    return nc


def _get_program():
    if "nc" not in _cache:
        _cache["nc"] = _build_program()
        _cache["wts"] = _weights()
    return _cache["nc"], _cache["wts"]


def _shard_inputs(x0_pred):
    x = np.ascontiguousarray(np.asarray(x0_pred, dtype=np.float32))
    _, wts = _get_program()
    in_maps = []
    for i in range(NCORES):
        shard = x[i * BPC:(i + 1) * BPC]                      # [16,2,256,256]
        arr = shard.reshape(BPC, 2, 2, 128, P).transpose(3, 1, 2, 0, 4)
        in_maps.append({"xin": np.ascontiguousarray(arr), "wts": wts})
    return in_maps


def _unshard(results):
    outs = []
    for i in range(NCORES):
        y = results[i]["yout"]                                # [128,2,16,256]
        outs.append(y.transpose(2, 1, 0, 3).reshape(BPC, 1, P, P))
    return np.ascontiguousarray(np.concatenate(outs, axis=0))


def _run(x0_pred, trace=False, tmpdir=None):
    from concourse.bass_utils import run_bass_kernel_spmd
    nc, _ = _get_program()
    in_maps = _shard_inputs(x0_pred)
    res = run_bass_kernel_spmd(nc, in_maps, list(range(NCORES)),
                               trace=trace, tmpdir=tmpdir)
    return _unshard(res.results), res


def kernel(x0_pred):
    out, _ = _run(x0_pred, trace=False)
    return out


# revision 8
# speedup vs baseline: 1.3178x; 1.3178x over previous
"""Trainium2 Bass kernel for nn_DarcyResidual (P=256, B=128, 8 NeuronCores).

Math (reference):
    a = (x0 + 1.5) / 0.2,  p = (x1 + 0.9) / 115
    residual = -a*(p_d00 + p_d11) - a_d0*p_d0 - a_d1*p_d1 - 1
2nd-order central differences inside, 2nd-order one-sided at borders,
h = 1/256 on both axes.

Folded form computed here (G = 5/(460 h^2)):
    residual = -G * [ (X0 + 1.5)*U4 + S1*R1 + C1a*C1p ] - 1
      U4  = 4*(rowD2raw(X1) + colD2raw(X1))   (raw h^2-scaled 2nd diffs)
      R1  = rowD1raw(X1), S1 = rowD1raw(X0)   (raw 2h-scaled 1st diffs)
      C1p = colD1raw(X1), C1a = colD1raw(X0)

Layout per core (16 images): SBUF [partition = row-within-128-block,
free = (row-block k:2, image b, col j:256)].  Row (d0) derivatives are
TensorE matmuls with banded stencil matrices in fp32r (boundary rows are
rows of the matrices; the -2I of the column stencil is folded in as
W_R2 = 4*(D2 - 2I)).  Column (d1) stencils are shifted-AP DVE ops.  The
two small gradient-product terms are written as bf16 and summed in PSUM
via identity-matmul accumulation.  Border columns j=0,255 get their own
small one-sided pipeline.  ScalarE does PSUM evacuation + final affine.
"""

import numpy as np

P = 256
B = 128
NCORES = 8
BPC = B // NCORES          # images per core = 16
CHUNKS = 4
BCH = BPC // CHUNKS        # images per chunk = 4
FCH = 2 * BCH * P          # chunk free size = 2048
GAMMA = 5.0 * 65536.0 / 460.0

_cache = {}


def _weights():
    """[128, 12, 128] stacked lhsT blocks for the row-stencil matmuls."""
    D1 = np.zeros((P, P), dtype=np.float64)
    for i in range(1, P - 1):
        D1[i, i - 1] = -1.0
        D1[i, i + 1] = 1.0
    D1[0, 0:3] = [-3.0, 4.0, -1.0]
    D1[P - 1, P - 3:P] = [1.0, -4.0, 3.0]

    D2 = np.zeros((P, P), dtype=np.float64)
    for i in range(1, P - 1):
        D2[i, i - 1] = 1.0
        D2[i, i] = -2.0
        D2[i, i + 1] = 1.0
    D2[0, 0:4] = [2.0, -5.0, 4.0, -1.0]
    D2[P - 1, P - 4:P] = [-1.0, 4.0, -5.0, 2.0]

    WR2 = 4.0 * (D2 - 2.0 * np.eye(P))   # main (col -2I folded here)
    WR2E = 4.0 * D2                      # edge (col stencil complete)

    wts = np.zeros((128, 12, 128), dtype=np.float32)
    for m in range(2):
        for kb in range(2):
            i = m * 2 + kb
            blk = lambda W: W[m * 128:(m + 1) * 128, kb * 128:(kb + 1) * 128].T
            wts[:, i, :] = blk(D1)
            wts[:, 4 + i, :] = blk(WR2)
            wts[:, 8 + i, :] = blk(WR2E)
    return wts


def _build_program():
    from concourse import bacc
    import concourse.mybir as mybir
    from concourse.tile import TileContext
    import ml_dtypes

    f32 = mybir.dt.float32
    f32r = mybir.dt.float32r
    bf16 = mybir.dt.bfloat16
    ADD = mybir.AluOpType.add
    SUB = mybir.AluOpType.subtract
    MUL = mybir.AluOpType.mult
    COPY = mybir.ActivationFunctionType.Copy

    nc = bacc.Bacc("TRN2", target_bir_lowering=False, debug=False,
                   num_devices=NCORES)
    xin = nc.dram_tensor("xin", [128, 2, 2, BPC, P], f32r, kind="ExternalInput")
    wts = nc.dram_tensor("wts", [128, 12, 128], f32r, kind="ExternalInput")
    ident = nc.dram_tensor("ident", [128, 128], bf16, kind="ExternalInput")
    yout = nc.dram_tensor("yout", [128, 2, BPC, P], f32, kind="ExternalOutput")

    with TileContext(nc) as tc:
        with (
            tc.tile_pool(name="const", bufs=1) as cpool,
            tc.tile_pool(name="edge", bufs=1) as epool,
            tc.tile_pool(name="work", bufs=2) as pool,
        ):
            wt = cpool.tile([128, 12, 128], f32r)
            nc.sync.dma_start(out=wt[:], in_=wts[:])
            ibf = cpool.tile([128, 128], bf16)
            nc.sync.dma_start(out=ibf[:], in_=ident[:])

            def W(i):
                return wt[:, i, :]

            stt = nc.vector.scalar_tensor_tensor

            # ------------- edge pipeline (output cols j=0 and j=255) -------
            X0e = epool.tile([128, 2, BPC, 8], f32r)
            X1e = epool.tile([128, 2, BPC, 8], f32r)
            nc.sync.dma_start(out=X0e[:, :, :, 0:4], in_=xin[:, 0, :, :, 0:4])
            nc.sync.dma_start(out=X0e[:, :, :, 4:8], in_=xin[:, 0, :, :, P - 4:P])
            nc.sync.dma_start(out=X1e[:, :, :, 0:4], in_=xin[:, 1, :, :, 0:4])
            nc.sync.dma_start(out=X1e[:, :, :, 4:8], in_=xin[:, 1, :, :, P - 4:P])

            X0ef = X0e.rearrange("p k b c -> p (k b c)")
            X1ef = X1e.rearrange("p k b c -> p (k b c)")
            # fp32 views, [128, 32, 8]
            E1 = X1e.bitcast(f32).rearrange("p k b c -> p (k b) c")
            E0 = X0e.bitcast(f32).rearrange("p k b c -> p (k b) c")

            def et(name, d=2):
                return epool.tile([128, 2 * BPC, d], f32, name=name, tag=name)

            with tc.tile_pool(name="psum_e", bufs=1, space="PSUM") as ppe:
                R2e = ppe.tile([128, 2, BPC, 8], f32)
                R1e = ppe.tile([128, 2, BPC, 8], f32)
                S1e = ppe.tile([128, 2, BPC, 8], f32)
                R2ef = R2e.rearrange("p k b c -> p (k b c)")
                R1ef = R1e.rearrange("p k b c -> p (k b c)")
                S1ef = S1e.rearrange("p k b c -> p (k b c)")
                for m in range(2):
                    osl = slice(m * 128, (m + 1) * 128)
                    for kb in range(2):
                        isl = slice(kb * 128, (kb + 1) * 128)
                        st, sp = kb == 0, kb == 1
                        nc.tensor.matmul(R1ef[:, osl], W(m * 2 + kb),
                                         X1ef[:, isl], start=st, stop=sp)
                        nc.tensor.matmul(S1ef[:, osl], W(m * 2 + kb),
                                         X0ef[:, isl], start=st, stop=sp)
                        nc.tensor.matmul(R2ef[:, osl], W(8 + m * 2 + kb),
                                         X1ef[:, isl], start=st, stop=sp)

                # paired forward/mirrored diffs: half 0 = j=0 side (fwd),
                # half 1 = j=255 side (also forward-oriented: f7-f6 etc.)
                a1, b1, c1 = et("a1"), et("b1"), et("c1")
                a0, b0 = et("a0"), et("b0")
                nc.vector.tensor_sub(a1[:], E1[:, :, 1:8:6], E1[:, :, 0:7:6])
                nc.vector.tensor_sub(b1[:], E1[:, :, 2:7:4], E1[:, :, 1:6:4])
                nc.vector.tensor_sub(c1[:], E1[:, :, 3:6:2], E1[:, :, 2:5:2])
                nc.vector.tensor_sub(a0[:], E0[:, :, 1:8:6], E0[:, :, 0:7:6])
                nc.vector.tensor_sub(b0[:], E0[:, :, 2:7:4], E0[:, :, 1:6:4])

                # one-sided raw stencils (Z sign flips on the mirror half)
                q, Z = et("q"), et("Z")
                C1pe, C1ae = et("C1pe"), et("C1ae")
                stt(q[:], b1[:], 3.0, c1[:], MUL, SUB)      # 3b - c
                stt(Z[:], a1[:], -2.0, q[:], MUL, ADD)      # -2a + 3b - c
                stt(C1pe[:], a1[:], 3.0, b1[:], MUL, SUB)   # 3a - b
                stt(C1ae[:], a0[:], 3.0, b0[:], MUL, SUB)

                RP2 = R2e.rearrange("p k b c -> p (k b) c")
                RP1 = R1e.rearrange("p k b c -> p (k b) c")
                U4e, tme, t2e = et("U4e"), et("tme"), et("t2e")
                stt(U4e[:, :, 0:1], Z[:, :, 0:1], 4.0, RP2[:, :, 0:1], MUL, ADD)
                stt(U4e[:, :, 1:2], Z[:, :, 1:2], -4.0, RP2[:, :, 7:8], MUL, ADD)

                Scpe = epool.tile([128, 2, BPC, 8], f32)
                nc.scalar.copy(out=Scpe.rearrange("p k b c -> p (k b c)"),
                               in_=S1ef[:])
                SP = Scpe.rearrange("p k b c -> p (k b) c")

                stt(tme[:], E0[:, :, 0:8:7], 1.5, U4e[:], ADD, MUL)
                nc.vector.tensor_mul(t2e[:], SP[:, :, 0:8:7], RP1[:, :, 0:8:7])
                nc.vector.tensor_add(tme[:], tme[:], t2e[:])
                nc.vector.tensor_mul(C1ae[:], C1ae[:], C1pe[:])  # t3e in-place
                nc.vector.tensor_add(tme[:], tme[:], C1ae[:])
                rese = epool.tile([128, 2 * BPC, 2], f32)
                nc.scalar.activation(rese[:], tme[:], COPY,
                                     bias=-1.0, scale=-GAMMA)

            # ------------- main pipeline, 4 chunks of 4 images -------------
            with tc.tile_pool(name="psum", bufs=2, space="PSUM") as pp:
                for c in range(CHUNKS):
                    b0c = c * BCH
                    X0c = pool.tile([128, 2, BCH, P], f32r, tag="x0")
                    X1c = pool.tile([128, 2, BCH, P], f32r, tag="x1")
                    nc.sync.dma_start(out=X0c[:], in_=xin[:, 0, :, b0c:b0c + BCH, :])
                    nc.sync.dma_start(out=X1c[:], in_=xin[:, 1, :, b0c:b0c + BCH, :])
                    X0rf = X0c.rearrange("p k b j -> p (k b j)")
                    X1rf = X1c.rearrange("p k b j -> p (k b j)")
                    X0f = X0c.bitcast(f32).rearrange("p k b j -> p (k b j)")
                    X1f = X1c.bitcast(f32).rearrange("p k b j -> p (k b j)")

                    u2 = pool.tile([128, FCH], f32, tag="u2")
                    C1p = pool.tile([128, FCH], bf16, tag="c1p")
                    C1a = pool.tile([128, FCH], bf16, tag="c1a")
                    scp = pool.tile([128, FCH], f32, tag="scp")
                    rcp = pool.tile([128, FCH], f32, tag="rcp")
                    t2b = pool.tile([128, FCH], bf16, tag="t2b")
                    t3b = pool.tile([128, FCH], bf16, tag="t3b")
                    tm = pool.tile([128, 2, BCH, P], f32, tag="tm")
                    tmf = tm.rearrange("p k b j -> p (k b j)")

                    # column stencils (interior; border cols handled above)
                    nc.vector.tensor_add(u2[:, 1:FCH - 1], X1f[:, 2:FCH],
                                         X1f[:, 0:FCH - 2])
                    nc.vector.tensor_sub(C1p[:, 1:FCH - 1], X1f[:, 2:FCH],
                                         X1f[:, 0:FCH - 2])
                    nc.vector.tensor_sub(C1a[:, 1:FCH - 1], X0f[:, 2:FCH],
                                         X0f[:, 0:FCH - 2])
                    nc.vector.tensor_mul(t3b[:], C1a[:], C1p[:])

                    for m in range(2):
                        for bp in range(2):
                            lo = m * (BCH * P) + bp * (2 * P)
                            sl = slice(lo, lo + 2 * P)
                            R2 = pp.tile([128, 2 * P], f32, tag="r2")
                            R1 = pp.tile([128, 2 * P], f32, tag="r1")
                            S1 = pp.tile([128, 2 * P], f32, tag="s1")
                            for kb in range(2):
                                ilo = kb * (BCH * P) + bp * (2 * P)
                                isl = slice(ilo, ilo + 2 * P)
                                st, sp = kb == 0, kb == 1
                                nc.tensor.matmul(R1[:], W(m * 2 + kb),
                                                 X1rf[:, isl], start=st, stop=sp)
                                nc.tensor.matmul(S1[:], W(m * 2 + kb),
                                                 X0rf[:, isl], start=st, stop=sp)
                                nc.tensor.matmul(R2[:], W(4 + m * 2 + kb),
                                                 X1rf[:, isl], start=st, stop=sp)
                            nc.scalar.copy(out=scp[:, sl], in_=S1[:])
                            nc.scalar.copy(out=rcp[:, sl], in_=R1[:])
                            # U4 in-place on u2
                            stt(u2[:, sl], u2[:, sl], 4.0, R2[:], MUL, ADD)

                    nc.vector.tensor_mul(t2b[:], scp[:], rcp[:])
                    # tm = (X0 + 1.5) * U4
                    stt(tmf[:], X0f[:], 1.5, u2[:], ADD, MUL)

                    for s in range(4):
                        sl = slice(s * 2 * P, (s + 1) * 2 * P)
                        ACC = pp.tile([128, 2 * P], f32, tag="acc")
                        nc.tensor.matmul(ACC[:], ibf[:], t2b[:, sl],
                                         start=True, stop=False)
                        nc.tensor.matmul(ACC[:], ibf[:], t3b[:, sl],
                                         start=False, stop=True)
                        nc.vector.tensor_add(tmf[:, sl], tmf[:, sl], ACC[:])

                    resv = tm.rearrange("p k b j -> p (k b) j")[:, :, 1:P - 1]
                    nc.scalar.activation(resv, resv, COPY,
                                         bias=-1.0, scale=-GAMMA)

                    # border columns from the edge pipeline
                    for e, j in ((0, 0), (1, P - 1)):
                        src = (rese.rearrange("p (k b) e -> p k b e", k=2)
                               [:, :, b0c:b0c + BCH, e:e + 1])
                        nc.scalar.copy(out=tm[:, :, :, j:j + 1], in_=src)

                    nc.sync.dma_start(out=yout[:, :, b0c:b0c + BCH, :], in_=tm[:])

    nc.compile()
    return nc


def _get_program():
    if "nc" not in _cache:
        _cache["nc"] = _build_program()
        _cache["wts"] = _weights()
        import ml_dtypes
        _cache["ident"] = np.eye(128, dtype=ml_dtypes.bfloat16)
    return _cache["nc"], _cache["wts"], _cache["ident"]


def _shard_inputs(x0_pred):
    x = np.ascontiguousarray(np.asarray(x0_pred, dtype=np.float32))
    _, wts, ident = _get_program()
    in_maps = []
    for i in range(NCORES):
        shard = x[i * BPC:(i + 1) * BPC]                      # [16,2,256,256]
        arr = shard.reshape(BPC, 2, 2, 128, P).transpose(3, 1, 2, 0, 4)
        in_maps.append({"xin": np.ascontiguousarray(arr), "wts": wts,
                        "ident": ident})
    return in_maps


def _unshard(results):
    outs = []
    for i in range(NCORES):
        y = results[i]["yout"]                                # [128,2,16,256]
        outs.append(y.transpose(2, 1, 0, 3).reshape(BPC, 1, P, P))
    return np.ascontiguousarray(np.concatenate(outs, axis=0))


def _run(x0_pred, trace=False, tmpdir=None):
    from concourse.bass_utils import run_bass_kernel_spmd
    nc = _get_program()[0]
    in_maps = _shard_inputs(x0_pred)
    res = run_bass_kernel_spmd(nc, in_maps, list(range(NCORES)),
                               trace=trace, tmpdir=tmpdir)
    return _unshard(res.results), res


def kernel(x0_pred):
    out, _ = _run(x0_pred, trace=False)
    return out
